# revision 1
# baseline (speedup 1.0000x reference)
"""Multi-head attention (B=4, S=2048, D=1024, H=16, Dk=64) on 8 trn2 NeuronCores.

Sharding: core = (batch b, head-group g) with b in 0..3, g in 0..1.
Each core computes attention for its batch and its 8 heads, plus the partial
out-projection for its 512 columns of Wo.  Host sums the two partials per
batch and adds bo.

Per-core kernel (matmuls in float32r = TF32 fast mode, ~4e-4 rel err):
  phase A: PE-transpose q/k/v 128x128 tiles (f32r transpose mode); project to
           qhT/khT [c=512, s=2048] (c on partitions, pairs of heads per
           128-partition tile) and vh [s=2048, c] stored with a ones column
           per head ([128, 16, 8, 65] layout).  Biases are folded in as K=1
           outer-product matmuls opening each accumulation group.
  phase B (per 1024-wide query chunk, per head):
           scoresT[sk,sq] = khT_h^T @ qhT_h  (K=64 contraction)
           probsT = exp(scoresT/8 + maskbias[sk])   (mask folded into the
           ACT per-partition bias; masked keys underflow to exactly 0)
           attnT[c(+sums),sq] += vh_ext^T @ probsT  (ones column gives the
           softmax denominator in row 64 for free)
           normalize: approx-reciprocal of row 64, replicate across 64
           partitions with a K=1 outer-product matmul, multiply -> concatT
  phase C (interleaved per query chunk, after its 8 heads finish):
           out[sq,:] = concatT^T @ Wo  (accumulate 4 c-chunks in PSUM)
"""

import os
import sys

sys.path.insert(0, "/opt/trn_rl_repo")

import numpy as np

B, S, D, H, DK = 4, 2048, 1024, 16, 64
CPG = 512          # projection columns per core (8 heads x 64)
NCORES = 8

_cache = {}


def _build_nc():
    import concourse.bass as bass
    import concourse.tile as tile
    from concourse import bacc, mybir

    f32 = mybir.dt.float32
    R = mybir.dt.float32r
    Exp = mybir.ActivationFunctionType.Exp

    nc = bacc.Bacc("TRN2", target_bir_lowering=False, debug=False)

    q_d = nc.dram_tensor("q", [S, D], f32, kind="ExternalInput").ap()
    k_d = nc.dram_tensor("k", [S, D], f32, kind="ExternalInput").ap()
    v_d = nc.dram_tensor("v", [S, D], f32, kind="ExternalInput").ap()
    wq_d = nc.dram_tensor("wq", [D, CPG], R, kind="ExternalInput").ap()
    wk_d = nc.dram_tensor("wk", [D, CPG], R, kind="ExternalInput").ap()
    wv_d = nc.dram_tensor("wv", [D, CPG], R, kind="ExternalInput").ap()
    wo_d = nc.dram_tensor("wo", [CPG, D], R, kind="ExternalInput").ap()
    bq_d = nc.dram_tensor("bq", [CPG], R, kind="ExternalInput").ap()
    bk_d = nc.dram_tensor("bk", [CPG], R, kind="ExternalInput").ap()
    bv_d = nc.dram_tensor("bv", [CPG], R, kind="ExternalInput").ap()
    mb_d = nc.dram_tensor("maskbias", [128, 16], f32, kind="ExternalInput").ap()
    ones_d = nc.dram_tensor("ones", [128, 512], R, kind="ExternalInput").ap()
    ident_d = nc.dram_tensor("ident", [128, 128], f32, kind="ExternalInput").ap()
    out_d = nc.dram_tensor("out", [S, D], f32, kind="ExternalOutput").ap()

    NSQ = S // 512       # 4 query/key 512-blocks
    NDCH = D // 128      # 8 contraction chunks for projections
    NSK = S // 128       # 16 key chunks
    NPAIR = 4            # head pairs per core

    with tile.TileContext(nc) as tc:
        import contextlib

        with contextlib.ExitStack() as ctx:
            # ---------- persistent tensors + constants ----------
            persist = ctx.enter_context(tc.tile_pool(name="persist", bufs=1))
            consts = ctx.enter_context(tc.tile_pool(name="consts", bufs=1))

            qhT_sb = persist.tile([128, NPAIR, S], R)   # [c%128, pair, sq]
            khT_sb = persist.tile([128, NPAIR, S], R)
            vh_sb = persist.tile([128, NSK, 8, DK + 1], R)  # ones col at 64

            ones_sb = consts.tile([1, 512], R)
            nc.sync.dma_start(out=ones_sb, in_=ones_d[0:1, :])
            nc.sync.dma_start(
                out=vh_sb[:, :, :, DK],
                in_=ones_d[:, 0:128].rearrange("p (a b) -> p a b", a=16),
            )
            mb_sb = consts.tile([128, 16], f32)
            nc.sync.dma_start(out=mb_sb, in_=mb_d)
            wo_sb = consts.tile([128, NPAIR, D], R)
            for j in range(NPAIR):
                nc.sync.dma_start(
                    out=wo_sb[:, j, :], in_=wo_d[j * 128 : j * 128 + 128, :]
                )

            # ---------- phase A: transposes + projections ----------
            with contextlib.ExitStack() as actx:
                aconsts = actx.enter_context(tc.tile_pool(name="aconsts", bufs=1))
                wpool = actx.enter_context(tc.tile_pool(name="wpool", bufs=2))
                natpool = actx.enter_context(tc.tile_pool(name="natpool", bufs=6))
                xtpool = actx.enter_context(tc.tile_pool(name="xtpool", bufs=3))
                tppool = actx.enter_context(
                    tc.tile_pool(name="tppool", bufs=2, space="PSUM")
                )
                prpool = actx.enter_context(
                    tc.tile_pool(name="prpool", bufs=4, space="PSUM")
                )

                ident = aconsts.tile([128, 128], f32)
                nc.sync.dma_start(out=ident, in_=ident_d)
                bq_sb = aconsts.tile([1, CPG], R)
                nc.sync.dma_start(out=bq_sb, in_=bq_d[None, :])
                bk_sb = aconsts.tile([1, CPG], R)
                nc.sync.dma_start(out=bk_sb, in_=bk_d[None, :])
                bv_sb = aconsts.tile([1, CPG], R)
                nc.sync.dma_start(out=bv_sb, in_=bv_d[None, :])

                for tname, x_d, w_d, b_sb in (
                    ("q", q_d, wq_d, bq_sb),
                    ("k", k_d, wk_d, bk_sb),
                    ("v", v_d, wv_d, bv_sb),
                ):
                    w_sb = wpool.tile([128, NDCH, CPG], R, tag="w")
                    first_nats = []
                    if tname == "q":
                        for i in range(4):
                            x_nat = natpool.tile([128, D], f32, tag="nat")
                            nc.sync.dma_start(out=x_nat, in_=x_d[i * 128 : i * 128 + 128, :])
                            first_nats.append(x_nat)
                    for j in range(NDCH):
                        nc.sync.dma_start(
                            out=w_sb[:, j, :], in_=w_d[j * 128 : j * 128 + 128, :]
                        )
                    for sq in range(NSQ):
                        if sq == 0 and first_nats:
                            nats = first_nats
                        else:
                            nats = []
                            for i in range(4):
                                x_nat = natpool.tile([128, D], f32, tag="nat")
                                r0 = sq * 512 + i * 128
                                nc.sync.dma_start(out=x_nat, in_=x_d[r0 : r0 + 128, :])
                                nats.append(x_nat)

                        # open accumulation groups: bias outer-product first
                        prs = []
                        for cch in range(4):
                            pr = prpool.tile([128, 512], f32, tag="pr")
                            prs.append(pr)
                            if tname == "v":
                                nc.tensor.matmul(
                                    pr,
                                    lhsT=ones_sb[0:1, 0:128],
                                    rhs=b_sb[0:1, :],
                                    start=True,
                                    stop=False,
                                )
                            else:
                                nc.tensor.matmul(
                                    pr,
                                    lhsT=b_sb[0:1, cch * 128 : cch * 128 + 128],
                                    rhs=ones_sb[0:1, 0:512],
                                    start=True,
                                    stop=False,
                                )

                        for j in range(NDCH):
                            tp = tppool.tile([128, 512], f32, tag="tp")
                            for i in range(4):
                                nc.tensor.transpose(
                                    out=tp[:, i * 128 : i * 128 + 128],
                                    in_=nats[i][:, j * 128 : j * 128 + 128],
                                    identity=ident,
                                )
                            xt = xtpool.tile([128, 512], R, tag="xt")
                            nc.scalar.copy(out=xt, in_=tp)
                            for cch in range(4):
                                if tname == "v":
                                    # vh[sk,c]: lhsT = xT chunk, rhs = W chunk
                                    nc.tensor.matmul(
                                        prs[cch],
                                        lhsT=xt[:, cch * 128 : cch * 128 + 128],
                                        rhs=w_sb[:, j, :],
                                        start=False,
                                        stop=(j == NDCH - 1),
                                    )
                                else:
                                    # qhT[c,sq]: lhsT = W chunk, rhs = xT
                                    nc.tensor.matmul(
                                        prs[cch],
                                        lhsT=w_sb[:, j, cch * 128 : cch * 128 + 128],
                                        rhs=xt,
                                        start=False,
                                        stop=(j == NDCH - 1),
                                    )

                        for cch in range(4):
                            if tname == "v":
                                skc = sq * 4 + cch
                                nc.vector.tensor_copy(
                                    out=vh_sb[:, skc, :, 0:DK],
                                    in_=prs[cch].rearrange("p (h d) -> p h d", h=8),
                                )
                            else:
                                dst = qhT_sb if tname == "q" else khT_sb
                                nc.vector.tensor_copy(
                                    out=dst[:, cch, sq * 512 : sq * 512 + 512],
                                    in_=prs[cch],
                                )

            # ---------- phase B: attention ----------
            concpool = ctx.enter_context(tc.tile_pool(name="concpool", bufs=1))
            concatT_sb = concpool.tile([128, NPAIR, S], R)
            with contextlib.ExitStack() as bctx:
                probpool = bctx.enter_context(tc.tile_pool(name="probpool", bufs=3))
                npool = bctx.enter_context(tc.tile_pool(name="npool", bufs=2))
                rppool = bctx.enter_context(tc.tile_pool(name="rppool", bufs=1))
                scpool = bctx.enter_context(
                    tc.tile_pool(name="scpool", bufs=2, space="PSUM")
                )
                atpool = bctx.enter_context(
                    tc.tile_pool(name="atpool", bufs=3, space="PSUM")
                )
                reppool = bctx.enter_context(
                    tc.tile_pool(name="reppool", bufs=1, space="PSUM")
                )

                for sq2 in range(S // 1024):
                    for pair in range(NPAIR):
                        for hh in range(2):
                            h = pair * 2 + hh
                            base = hh * 64
                            at_ps0 = atpool.tile([128, 512], f32, tag="at")
                            at_ps1 = atpool.tile([128, 512], f32, tag="at")
                            at_halves = (at_ps0, at_ps1)
                            for sk in range(NSK):
                                sc_ps = scpool.tile([128, 1024], f32, tag="sc")
                                for half in range(2):
                                    qoff = sq2 * 1024 + half * 512
                                    nc.tensor.matmul(
                                        sc_ps[:, half * 512 : half * 512 + 512],
                                        lhsT=khT_sb[
                                            base : base + 64,
                                            pair,
                                            sk * 128 : sk * 128 + 128,
                                        ],
                                        rhs=qhT_sb[
                                            base : base + 64, pair, qoff : qoff + 512
                                        ],
                                        start=True,
                                        stop=True,
                                    )
                                probs = probpool.tile([128, 1024], R, tag="probs")
                                nc.scalar.activation(
                                    out=probs,
                                    in_=sc_ps,
                                    func=Exp,
                                    bias=mb_sb[:, sk : sk + 1],
                                    scale=0.125,
                                )
                                for half in range(2):
                                    nc.tensor.matmul(
                                        at_halves[half][0:65, :],
                                        lhsT=vh_sb[:, sk, h, :],
                                        rhs=probs[:, half * 512 : half * 512 + 512],
                                        start=(sk == 0),
                                        stop=(sk == NSK - 1),
                                    )
                            attn_sb = npool.tile([128, 1024], f32, tag="attn")
                            for half in range(2):
                                nc.vector.tensor_copy(
                                    out=attn_sb[
                                        0:65, half * 512 : half * 512 + 512
                                    ],
                                    in_=at_halves[half][0:65, :],
                                )
                            recip32 = rppool.tile([1, 1024], f32, tag="recip32")
                            nc.vector.reciprocal(recip32, attn_sb[64:65, :])
                            recip = rppool.tile([1, 1024], R, tag="recip")
                            nc.vector.tensor_copy(out=recip, in_=recip32)
                            for half in range(2):
                                rep_ps = reppool.tile([64, 512], f32, tag="rep")
                                nc.tensor.matmul(
                                    rep_ps,
                                    lhsT=ones_sb[0:1, 0:64],
                                    rhs=recip[0:1, half * 512 : half * 512 + 512],
                                    start=True,
                                    stop=True,
                                )
                                nc.vector.tensor_mul(
                                    concatT_sb[
                                        base : base + 64,
                                        pair,
                                        sq2 * 1024 + half * 512 : sq2 * 1024
                                        + half * 512
                                        + 512,
                                    ],
                                    attn_sb[0:64, half * 512 : half * 512 + 512],
                                    rep_ps,
                                )

            # ---------- phase C: out projection ----------
            with contextlib.ExitStack() as cctx:
                outpool = cctx.enter_context(
                    tc.tile_pool(name="outpool", bufs=3)
                )
                opspool = cctx.enter_context(
                    tc.tile_pool(name="opspool", bufs=4, space="PSUM")
                )
                for sqc in range(S // 128):
                    for do in range(2):
                        o_ps = opspool.tile([128, 512], f32, tag="ops")
                        for j in range(NPAIR):
                            nc.tensor.matmul(
                                o_ps,
                                lhsT=concatT_sb[
                                    :, j, sqc * 128 : sqc * 128 + 128
                                ],
                                rhs=wo_sb[:, j, do * 512 : do * 512 + 512],
                                start=(j == 0),
                                stop=(j == NPAIR - 1),
                            )
                        o_sb = outpool.tile([128, 512], f32, tag="osb")
                        nc.vector.tensor_copy(out=o_sb, in_=o_ps)
                        nc.sync.dma_start(
                            out=out_d[
                                sqc * 128 : sqc * 128 + 128,
                                do * 512 : do * 512 + 512,
                            ],
                            in_=o_sb,
                        )

    nc.compile()
    return nc


def get_nc():
    if "nc" not in _cache:
        _cache["nc"] = _build_nc()
    return _cache["nc"]


def make_in_maps(q, k, v, mask, Wq, bq, Wk, bk, Wv, bv, Wo, bo):
    f32 = np.float32
    c = np.ascontiguousarray
    in_maps = []
    for core in range(NCORES):
        b, g = core // 2, core % 2
        cols = slice(g * CPG, (g + 1) * CPG)
        mb = (-1e9 * (1.0 - np.asarray(mask[b, 0], f32))).reshape(16, 128).T
        in_maps.append(
            {
                "q": c(np.asarray(q[b], f32)),
                "k": c(np.asarray(k[b], f32)),
                "v": c(np.asarray(v[b], f32)),
                "wq": c(np.asarray(Wq[:, cols], f32)),
                "wk": c(np.asarray(Wk[:, cols], f32)),
                "wv": c(np.asarray(Wv[:, cols], f32)),
                "wo": c(np.asarray(Wo[cols, :], f32)),
                "bq": c(np.asarray(bq[cols], f32)),
                "bk": c(np.asarray(bk[cols], f32)),
                "bv": c(np.asarray(bv[cols], f32)),
                "maskbias": c(mb),
                "ones": np.ones((128, 512), f32),
                "ident": np.eye(128, dtype=f32),
            }
        )
    return in_maps


def gather(results, bo):
    out = np.zeros((B, S, D), np.float32)
    for core in range(NCORES):
        b = core // 2
        out[b] += results[core]["out"]
    out += np.asarray(bo, np.float32)[None, None, :]
    return out


def run_on_hw(in_maps, trace=False, trace_cores=None):
    from concourse.bass_utils import run_bass_kernel_spmd

    nc = get_nc()
    return run_bass_kernel_spmd(
        nc,
        in_maps,
        list(range(NCORES)),
        trace=trace,
        trace_cores=trace_cores,
    )


def kernel(q, k, v, mask, Wq, bq, Wk, bk, Wv, bv, Wo, bo):
    in_maps = make_in_maps(q, k, v, mask, Wq, bq, Wk, bk, Wv, bv, Wo, bo)
    res = run_on_hw(in_maps)
    return gather(res.results, bo)



# revision 2
# speedup vs baseline: 2.2913x; 2.2913x over previous
"""Multi-head attention (B=4, S=2048, D=1024, H=16, Dk=64) on 8 trn2 NeuronCores.

Sharding: core = (batch b, head-group g), g selects 8 heads (512 proj cols).
Host sums the two partial out-projections per batch and adds bo.

Key optimizations over the v1 kernel (912us):
  * Host-side key compaction: masked keys give exactly-zero probs in the
    reference (exp(-1e9/8) underflows), so drop them on the host and pad
    k/v to SK=1152 rows (mask is Bernoulli(0.5), so ~1024 survive; fall
    back to SK=2048 if a batch ever exceeds 1152).  Cuts k/v projections,
    scores, attn and the scalar-engine exp work by ~44%.
  * bf16 matmuls everywhere (host pre-casts inputs/weights): same 1
    col/cycle stream rate as f32r but fast weight loads (FWL), half the
    DMA and SBUF footprint.  fp32 accumulation in PSUM.
  * Row-tiled scores: the K=64 scores matmuls of the two heads of a pair
    run concurrently in PE row groups 0/64 (tile_position auto-derived
    from the partition bases) -> 2x PE throughput on scores.
  * Pad-key handling via a per-chunk -1e9 activation bias (probs of pad
    keys are exactly 0), plus an indicator column in vh giving the
    softmax denominator for free (row 64 of the attn accumulator).
  * Normalization via replicate-denominator matmul + reciprocal_approx_fast
    on 64 partitions (the v1 kernel burned 6.5us per [1,1024] serial
    reciprocal).
  * Software-pipelined emission: the q projections for query blocks 1-3
    and the out-projection (phase C) are woven into the ACT-bound
    attention loop as PE filler so the tensor engine never idles (keeps
    the HAM clock gate at 2.4 GHz; the v1 kernel sat at 1.2 GHz for
    600us of its runtime).
"""

import sys

sys.path.insert(0, "/opt/trn_rl_repo")

import numpy as np

B, S, D, H, DK = 4, 2048, 1024, 16, 64
CPG = 512          # projection columns per core (8 heads x 64)
NCORES = 8
SK_FAST = 1152     # compacted+padded key rows (multiple of 128)

_cache = {}


def _build_nc(SK):
    import contextlib
    from collections import deque

    import concourse.bass as bass
    import concourse.tile as tile
    from concourse import bacc, mybir

    f32 = mybir.dt.float32
    R = mybir.dt.float32r
    BF = mybir.dt.bfloat16
    Exp = mybir.ActivationFunctionType.Exp

    NSK = SK // 128        # key chunks of 128
    NQB = S // 512         # query 512-blocks (4)
    NDCH = D // 128        # contraction chunks for projections (8)
    NPAIR = 4              # head pairs per core

    nc = bacc.Bacc("TRN2", target_bir_lowering=False, debug=False)

    q_d = nc.dram_tensor("q", [S, D], BF, kind="ExternalInput").ap()
    k_d = nc.dram_tensor("kc", [SK, D], BF, kind="ExternalInput").ap()
    v_d = nc.dram_tensor("vc", [SK, D], BF, kind="ExternalInput").ap()
    wq_d = nc.dram_tensor("wq", [D, CPG], BF, kind="ExternalInput").ap()
    wk_d = nc.dram_tensor("wk", [D, CPG], BF, kind="ExternalInput").ap()
    wv_d = nc.dram_tensor("wv", [D, CPG], BF, kind="ExternalInput").ap()
    wo_d = nc.dram_tensor("wo", [CPG, D], BF, kind="ExternalInput").ap()
    bq_d = nc.dram_tensor("bq", [CPG], BF, kind="ExternalInput").ap()
    bk_d = nc.dram_tensor("bk", [CPG], BF, kind="ExternalInput").ap()
    bv_d = nc.dram_tensor("bv", [CPG], BF, kind="ExternalInput").ap()
    mb_d = nc.dram_tensor("maskbias", [128, NSK], f32, kind="ExternalInput").ap()
    ki_d = nc.dram_tensor("keyind", [128, NSK * 8], BF, kind="ExternalInput").ap()
    ones_d = nc.dram_tensor("ones", [128, 512], BF, kind="ExternalInput").ap()
    onesr_d = nc.dram_tensor("onesr", [1, 64], R, kind="ExternalInput").ap()
    ident_d = nc.dram_tensor("ident", [128, 128], BF, kind="ExternalInput").ap()
    out_d = nc.dram_tensor("out", [S, D], f32, kind="ExternalOutput").ap()

    with tile.TileContext(nc) as tc:
        import contextlib

        with contextlib.ExitStack() as ctx:
            # ---------- persistent tensors + constants ----------
            persist = ctx.enter_context(tc.tile_pool(name="persist", bufs=1))
            consts = ctx.enter_context(tc.tile_pool(name="consts", bufs=1))

            qhT_sb = persist.tile([128, NPAIR, S], BF)     # [c%128, pair, sq]
            khT_sb = persist.tile([128, NPAIR, SK], BF)
            vh_sb = persist.tile([128, NSK, 8, DK + 1], BF)  # ind col at 64
            concatT_sb = persist.tile([128, NPAIR, S], BF)
            wq_sb = persist.tile([128, NDCH, CPG], BF)
            wk_sb = persist.tile([128, NDCH, CPG], BF)
            wv_sb = persist.tile([128, NDCH, CPG], BF)
            wo_sb = persist.tile([128, NPAIR, D], BF)

            ones_sb = consts.tile([1, 512], BF)
            onesr_sb = consts.tile([1, 64], R)
            ident = consts.tile([128, 128], BF)
            mb_sb = consts.tile([128, NSK], f32)
            bq_sb = consts.tile([1, CPG], BF)
            bk_sb = consts.tile([1, CPG], BF)
            bv_sb = consts.tile([1, CPG], BF)

            nc.sync.dma_start(out=ones_sb, in_=ones_d[0:1, :])
            nc.sync.dma_start(out=onesr_sb, in_=onesr_d)
            nc.sync.dma_start(out=ident, in_=ident_d)
            nc.sync.dma_start(out=mb_sb, in_=mb_d)
            nc.sync.dma_start(out=bq_sb, in_=bq_d[None, :])
            nc.sync.dma_start(out=bk_sb, in_=bk_d[None, :])
            nc.sync.dma_start(out=bv_sb, in_=bv_d[None, :])
            # indicator column of vh (softmax denominator counts real keys)
            nc.sync.dma_start(
                out=vh_sb[:, :, :, DK],
                in_=ki_d.rearrange("p (a b) -> p a b", a=NSK),
            )
            for t_sb, t_d in ((wk_sb, wk_d), (wv_sb, wv_d), (wq_sb, wq_d)):
                for j in range(NDCH):
                    nc.sync.dma_start(
                        out=t_sb[:, j, :], in_=t_d[j * 128 : j * 128 + 128, :]
                    )
            for j in range(NPAIR):
                nc.sync.dma_start(
                    out=wo_sb[:, j, :], in_=wo_d[j * 128 : j * 128 + 128, :]
                )

            # ---------- shared rings ----------
            # PSUM: sc 2x[128,1024]f32 (4 banks) + at 2x[128,512]f32 (2)
            #       + fill 2x[128,512] (2) = 8 banks exactly.
            psum = ctx.enter_context(tc.tile_pool(name="psum", bufs=2, space="PSUM"))
            natpool = ctx.enter_context(tc.tile_pool(name="natpool", bufs=8))
            xtpool = ctx.enter_context(tc.tile_pool(name="xtpool", bufs=10))
            probpool = ctx.enter_context(tc.tile_pool(name="probpool", bufs=3))
            smallpool = ctx.enter_context(tc.tile_pool(name="smallpool", bufs=4))
            outpool = ctx.enter_context(tc.tile_pool(name="outpool", bufs=3))

            # ---------- projection block emitters ----------
            def proj_block_units(kind, x_d, w_sb, b_sb, s0, w, act_copy):
                """Generate unit-closures for projecting x rows [s0, s0+w).
                kind: 'q'/'k' -> [c, s] into qhT_sb/khT_sb; 'v' -> vh_sb."""
                nsub = w // 128
                nats = []
                xts = []

                def u_load():
                    for i in range(nsub):
                        nat = natpool.tile([128, D], BF, tag="nat")
                        r0 = s0 + i * 128
                        nc.sync.dma_start(out=nat, in_=x_d[r0 : r0 + 128, :])
                        nats.append(nat)

                yield 0.1, u_load

                def u_tp(j):
                    def run():
                        tp = psum.tile([128, 512], BF, tag="fill")
                        for i in range(nsub):
                            nc.tensor.transpose(
                                out=tp[:, i * 128 : i * 128 + 128],
                                in_=nats[i][:, j * 128 : j * 128 + 128],
                                identity=ident,
                            )
                        xt = xtpool.tile([128, 512], BF, tag="xt")
                        if act_copy:
                            nc.scalar.copy(out=xt[:, :w], in_=tp[:, :w])
                        else:
                            nc.vector.tensor_copy(out=xt[:, :w], in_=tp[:, :w])
                        xts.append(xt)

                    return run

                for j in range(NDCH):
                    yield 0.3, u_tp(j)

                if kind in ("q", "k"):
                    dst = qhT_sb if kind == "q" else khT_sb

                    def u_proj(cch):
                        def run():
                            pr = psum.tile([128, 512], f32, tag="at")
                            nc.tensor.matmul(
                                pr[:, :w],
                                lhsT=b_sb[0:1, cch * 128 : cch * 128 + 128],
                                rhs=ones_sb[0:1, :w],
                                start=True,
                                stop=False,
                            )
                            for j in range(NDCH):
                                nc.tensor.matmul(
                                    pr[:, :w],
                                    lhsT=w_sb[:, j, cch * 128 : cch * 128 + 128],
                                    rhs=xts[j][:, :w],
                                    start=False,
                                    stop=(j == NDCH - 1),
                                )
                            nc.vector.tensor_copy(
                                out=dst[:, cch, s0 : s0 + w], in_=pr[:, :w]
                            )

                        return run

                    for cch in range(NPAIR):
                        yield 1.9, u_proj(cch)
                else:

                    def u_projv(sub):
                        def run():
                            pr = psum.tile([128, 512], f32, tag="at")
                            nc.tensor.matmul(
                                pr,
                                lhsT=ones_sb[0:1, 0:128],
                                rhs=b_sb[0:1, :],
                                start=True,
                                stop=False,
                            )
                            for j in range(NDCH):
                                nc.tensor.matmul(
                                    pr,
                                    lhsT=xts[j][:, sub * 128 : sub * 128 + 128],
                                    rhs=w_sb[:, j, :],
                                    start=False,
                                    stop=(j == NDCH - 1),
                                )
                            skc = (s0 + sub * 128) // 128
                            nc.vector.tensor_copy(
                                out=vh_sb[:, skc, :, 0:DK],
                                in_=pr.rearrange("p (h d) -> p h d", h=8),
                            )

                        return run

                    for sub in range(nsub):
                        yield 1.9, u_projv(sub)

            def phasec_units(qc):
                """Out-projection for query block qc (concatT -> out)."""

                def u_cblk(sqc, do):
                    def run():
                        o_ps = psum.tile([128, 512], f32, tag="fill")
                        for p in range(NPAIR):
                            nc.tensor.matmul(
                                o_ps,
                                lhsT=concatT_sb[
                                    :, p, sqc * 128 : sqc * 128 + 128
                                ],
                                rhs=wo_sb[:, p, do * 512 : do * 512 + 512],
                                start=(p == 0),
                                stop=(p == NPAIR - 1),
                            )
                        o_sb = outpool.tile([128, 512], f32, tag="osb")
                        nc.vector.tensor_copy(out=o_sb, in_=o_ps)
                        nc.sync.dma_start(
                            out=out_d[
                                sqc * 128 : sqc * 128 + 128,
                                do * 512 : do * 512 + 512,
                            ],
                            in_=o_sb,
                        )

                    return run

                for sq in range(4):
                    for do in range(2):
                        yield 1.0, u_cblk(qc * 4 + sq, do)

            def drain(units):
                for _, u in units:
                    u()

            # ---------- prefix: k, v and q block 0 (ACT does the copies) ----
            kblocks = []
            o = 0
            while o < SK:
                w = min(512, SK - o)
                kblocks.append((o, w))
                o += w
            for s0, w in kblocks:
                drain(proj_block_units("k", k_d, wk_sb, bk_sb, s0, w, True))
            for s0, w in kblocks:
                drain(proj_block_units("v", v_d, wv_sb, bv_sb, s0, w, True))
            drain(proj_block_units("q", q_d, wq_sb, bq_sb, 0, 512, True))

            # ---------- attention spine with woven fillers ----------
            fillers = deque()

            def weave(debt):
                while fillers and debt >= fillers[0][0]:
                    cost, u = fillers.popleft()
                    u()
                    debt -= cost
                return debt

            debt = 0.0
            for qc in range(NQB):
                if qc + 1 < NQB:
                    fillers.extend(
                        proj_block_units(
                            "q", q_d, wq_sb, bq_sb, (qc + 1) * 512, 512, False
                        )
                    )
                for pair in range(NPAIR):
                    at0 = psum.tile([128, 512], f32, tag="at")
                    at1 = psum.tile([128, 512], f32, tag="at")
                    ats = (at0, at1)
                    for skc in range(NSK):
                        sc = psum.tile([128, 1024], f32, tag="sc")
                        for hh in range(2):
                            nc.tensor.matmul(
                                sc[:, hh * 512 : hh * 512 + 512],
                                lhsT=khT_sb[
                                    hh * 64 : hh * 64 + 64,
                                    pair,
                                    skc * 128 : skc * 128 + 128,
                                ],
                                rhs=qhT_sb[
                                    hh * 64 : hh * 64 + 64,
                                    pair,
                                    qc * 512 : qc * 512 + 512,
                                ],
                                start=True,
                                stop=True,
                            )
                        probs = probpool.tile([128, 1024], BF, tag="probs")
                        nc.scalar.activation(
                            out=probs,
                            in_=sc,
                            func=Exp,
                            bias=mb_sb[:, skc : skc + 1],
                            scale=0.125,
                        )
                        for hh in range(2):
                            nc.tensor.matmul(
                                ats[hh][0:65, :],
                                lhsT=vh_sb[:, skc, pair * 2 + hh, :],
                                rhs=probs[:, hh * 512 : hh * 512 + 512],
                                start=(skc == 0),
                                stop=(skc == NSK - 1),
                            )
                        debt = weave(debt + 0.45)

                    # normalize: replicate denom, approx-reciprocal, multiply
                    rep = psum.tile([128, 1024], f32, tag="sc")
                    for hh in range(2):
                        dn = smallpool.tile([1, 512], R, tag="dn")
                        nc.vector.tensor_copy(out=dn, in_=ats[hh][64:65, :])
                        nc.tensor.matmul(
                            rep[0:64, hh * 512 : hh * 512 + 512],
                            lhsT=onesr_sb,
                            rhs=dn,
                            start=True,
                            stop=True,
                        )
                        rc = smallpool.tile([64, 512], f32, tag="rc")
                        nc.vector.reciprocal_approx_fast(
                            out=rc, in_=rep[0:64, hh * 512 : hh * 512 + 512]
                        )
                        nc.vector.tensor_mul(
                            concatT_sb[
                                hh * 64 : hh * 64 + 64,
                                pair,
                                qc * 512 : qc * 512 + 512,
                            ],
                            ats[hh][0:64, :],
                            rc,
                        )
                    debt = weave(debt + 1.0)
                fillers.extend(phasec_units(qc))
            # drain remaining fillers (last out-projection block)
            debt = weave(1e9)

    nc.compile()
    return nc


def get_nc(SK=SK_FAST):
    if SK not in _cache:
        _cache[SK] = _build_nc(SK)
    return _cache[SK]


def make_in_maps(q, k, v, mask, Wq, bq, Wk, bk, Wv, bv, Wo, bo):
    import ml_dtypes

    bf16 = ml_dtypes.bfloat16
    f32 = np.float32
    c = np.ascontiguousarray

    counts = [int(np.asarray(mask[b, 0]).sum()) for b in range(B)]
    SK = SK_FAST if max(counts) <= SK_FAST else S
    NSK = SK // 128

    grid = np.arange(128)[:, None] + 128 * np.arange(NSK)[None, :]  # [128,NSK]
    per_batch = []
    for b in range(B):
        idx = np.flatnonzero(np.asarray(mask[b, 0]))
        nk = len(idx)
        kc = np.zeros((SK, D), bf16)
        kc[:nk] = np.asarray(k[b], f32)[idx].astype(bf16)
        vc = np.zeros((SK, D), bf16)
        vc[:nk] = np.asarray(v[b], f32)[idx].astype(bf16)
        mb = np.where(grid < nk, 0.0, -1e9).astype(f32)
        ki = np.broadcast_to(
            (grid < nk).astype(bf16)[:, :, None], (128, NSK, 8)
        ).reshape(128, NSK * 8)
        per_batch.append(
            {
                "q": np.asarray(q[b], f32).astype(bf16),
                "kc": kc,
                "vc": vc,
                "maskbias": mb,
                "keyind": c(ki),
            }
        )

    ones = np.ones((128, 512), bf16)
    onesr = np.ones((1, 64), f32)
    ident = np.eye(128, dtype=bf16)
    in_maps = []
    for core in range(NCORES):
        b, g = core // 2, core % 2
        cols = slice(g * CPG, (g + 1) * CPG)
        m = dict(per_batch[b])
        m.update(
            {
                "wq": np.asarray(Wq[:, cols], f32).astype(bf16),
                "wk": np.asarray(Wk[:, cols], f32).astype(bf16),
                "wv": np.asarray(Wv[:, cols], f32).astype(bf16),
                "wo": np.asarray(Wo[cols, :], f32).astype(bf16),
                "bq": np.asarray(bq[cols], f32).astype(bf16),
                "bk": np.asarray(bk[cols], f32).astype(bf16),
                "bv": np.asarray(bv[cols], f32).astype(bf16),
                "ones": ones,
                "onesr": onesr,
                "ident": ident,
            }
        )
        in_maps.append(m)
    return in_maps, SK


def gather(results, bo):
    out = np.zeros((B, S, D), np.float32)
    for core in range(NCORES):
        b = core // 2
        out[b] += results[core]["out"]
    out += np.asarray(bo, np.float32)[None, None, :]
    return out


def run_on_hw(in_maps, SK=SK_FAST, trace=False, trace_cores=None):
    from concourse.bass_utils import run_bass_kernel_spmd

    nc = get_nc(SK)
    return run_bass_kernel_spmd(
        nc,
        in_maps,
        list(range(NCORES)),
        trace=trace,
        trace_cores=trace_cores,
    )


def kernel(q, k, v, mask, Wq, bq, Wk, bk, Wv, bv, Wo, bo):
    in_maps, SK = make_in_maps(q, k, v, mask, Wq, bq, Wk, bk, Wv, bv, Wo, bo)
    res = run_on_hw(in_maps, SK)
    return gather(res.results, bo)
